# revision 2
# baseline (speedup 1.0000x reference)
"""KANLayer kernel v3 for 8 Trainium2 NeuronCores.

Cost model of this environment (measured, see probe*.py):
  * every compute-engine instruction (op OR semaphore wait) costs ~20-50us
    fixed + bytes/(~40-80GB/s); sync-queue instructions ~3us; DMA bandwidth
    ~460GB/s (effectively free at our sizes);
  * >=3 concurrently-active compute engines serialize (par3 probe: PE+DVE+ACT
    independent = sum, not max), so total ~= sum of all engine instruction
    costs; minimizing TOTAL instructions+waits is the whole game;
  * the 192 f32r matmuls (N<=512 ISA cap) are a ~5.2ms floor.

v3 structure (per core, per NEFF iteration):
  * x2 = Square(s*x+t) directly from xt (not from xn) - kills the ACT
    xn->x2 retirement chain;
  * per-feature min/max: [128,2,4096] pair reduces placed AFTER the pair's
    x3 ops, where all their gating is already implied (zero waits);
  * scale/bias stats: ONE batched 4-op chain per iteration, consumed by the
    NEXT iteration (data is identical across NEFF repeats); iteration 0
    computes stats per-pair inline (s_st0);
  * spline coefficient sum: 3 chained DVE adds into the f32r tile the PE
    reads; that tile aliases out_all (dead after last matmul read);
  * c3 bias: c3nat is DMA'd into stageB after the adds consume it; one DVE
    reduce; folded into the single [128,4096] ACT Identity drain;
  * one out-store DMA, delayed one iteration so it never blocks producers.

Sharding: tensor-parallel over O. Core r owns output columns [128r, 128r+128).
"""

import numpy as np

import concourse.bass as bass
import concourse.mybir as mybir
from concourse.bass_utils import run_bass_kernel_spmd

P = 128
B = 4096
D = 1024
O = 1024
S = 4
KC = 4
NCORES = 8
OS = O // NCORES   # 128
DC = D // P        # 8
QW = 512
NQ = B // QW       # 8

F32 = mybir.dt.float32
F32R = mybir.dt.float32r
AX = mybir.AxisListType
ALU = mybir.AluOpType
ACTF = mybir.ActivationFunctionType

_CACHE = {}


def _build_bass(n_iters: int = 1, timing_mode: bool = False) -> bass.Bass:
    nc = bass.Bass(num_devices=NCORES)

    kind = {} if timing_mode else {"kind": "ExternalInput"}
    okind = {} if timing_mode else {"kind": "ExternalOutput"}
    xt = nc.dram_tensor("xt", [D, B], F32, **kind)
    # [S, P, KC*DC*OS]: plane s is one contiguous [128, 4096] DMA
    coeffs = nc.dram_tensor("coeffs", [S, P, KC * DC * OS], F32, **kind)
    c3nat = nc.dram_tensor("c3nat", [OS, D * S], F32, **kind)
    out_t = nc.dram_tensor("out_t", [OS, B], F32R, **okind)
    dummy = (
        nc.dram_tensor("tout", [P, 2], F32, kind="ExternalOutput")
        if timing_mode
        else None
    )

    from contextlib import ExitStack

    ctx = ExitStack()
    with ctx:
        sem = lambda name: ctx.enter_context(nc.semaphore(name))  # noqa: E731
        s_xte = sem("s_xte")  # +16 per even-chunk xt load (4/iter)
        s_xto = sem("s_xto")  # +16 per odd-chunk xt load (4/iter)
        s_ca = sem("s_ca")    # +16 per c_all plane-0 load
        s_cb = sem("s_cb")    # +16 per stageB plane load (3/iter)
        s_c3 = sem("s_c3")    # +16 per c3nat->stageB load
        s_out = sem("s_out")  # +16 per out store
        s_xn = sem("s_xn")    # +1 per ACT xn AND x2 op (16/iter)
        s_x3 = sem("s_x3")    # +1 per DVE x3 op (8/iter)
        s_rd = sem("s_rd")    # +1 per DVE pair-reduce op (8/iter)
        s_st = sem("s_st")    # +1 per iteration's stats batch
        s_st0 = sem("s_st0")  # iteration-0 per-pair stats groups (3 incs)
        s_ad = sem("s_ad")    # +1 per coeff add (3/iter)
        s_pe = sem("s_pe")    # +1 per finished matmul chunk (8/iter)
        s_dr = sem("s_dr")    # +1 per drain (1/iter)
        s_c3r = sem("s_c3r")  # +1 per c3 bias reduce
        s_dv = sem("s_dv")    # DVE intra-stats retirement chain (3/iter-ish)
        s_fin = sem("s_fin")  # timing-mode init / final store

        sb = lambda name, shape, dtype=F32: ctx.enter_context(  # noqa: E731
            nc.sbuf_tensor(name, shape, dtype)
        )
        xt_sb = sb("xt2", [P, 2 * B])           # 2 chunk slots, one tile
        xt_sl = [xt_sb[:, i * B : (i + 1) * B] for i in range(2)]
        xt_pair = xt_sb[:, :].rearrange("p (i b) -> p i b", i=2)
        xn_sb = [sb(f"xn{i}", [P, B], F32R) for i in range(2)]
        x2_sb = [sb(f"x2{i}", [P, B], F32R) for i in range(2)]
        x3_sb = [sb(f"x3{i}", [P, B], F32R) for i in range(2)]
        c_all = sb("c_all", [P, KC * DC * OS])  # f32 staging (DVE-only)
        stageB = sb("stageB", [P, KC * DC * OS])  # planes 1-3, then c3nat
        # out_all doubles as the f32r coefficient tile the PE reads (c_v)
        out_all = sb("out_all", [P, B], F32R)
        mn_all = sb("mn_all", [P, DC])
        mx_all = sb("mx_all", [P, DC])
        rng_all = sb("rng_all", [P, DC])
        s_all = sb("s_all", [P, DC])
        t0_all = sb("t0_all", [P, DC])
        t_all = sb("t_all", [P, DC])
        bias_sb = sb("bias_sb", [P, 2])         # ping-pong per iteration

        psum = ctx.enter_context(nc.psum_tensor("ps", [P, B], F32))

        c_v = out_all[:, :].rearrange("p (k j o) -> p k j o", k=KC, j=DC)

        NI = n_iters

        with nc.Block() as block:

            @block.sync
            def _(sp):
                if timing_mode:
                    sp.wait_ge(s_fin, 1)  # c_all memset done
                    zsrc = c_all[:, :]
                    sp.dma_start(
                        out=xt[:, :].rearrange("(n p) f -> p n f", p=P),
                        in_=bass.AP(
                            tensor=zsrc.tensor,
                            offset=zsrc.offset,
                            ap=[[zsrc.ap[0][0], P], [0, D // P], [1, B]],
                        ),
                    ).then_inc(s_fin, 16)
                    for s in range(S):
                        sp.wait_ge(s_fin, 17 + 16 * s)
                        sp.dma_start(out=coeffs[s], in_=zsrc).then_inc(s_fin, 16)
                    sp.wait_ge(s_fin, 81)
                    sp.dma_start(out=c3nat[:, :], in_=zsrc).then_inc(s_fin, 16)
                    sp.wait_ge(s_fin, 97)
                for it in range(NI):

                    def xt_load(j):
                        g = 8 * it + j
                        if g >= 2:
                            # slot holds chunk g-2: gate on its pair-reduce
                            # AND its xn (x2 reads xt; xn retirement explicit)
                            pr = (
                                8 * it + 2 * ((j - 2) // 2 + 1)
                                if j >= 2
                                else 8 * it
                            )
                            sp.wait_ge(s_rd, pr)
                            sp.wait_ge(s_xn, 2 * (g - 1))
                        sp.dma_start(
                            out=xt_sl[j % 2], in_=xt[j * P : (j + 1) * P, :]
                        ).then_inc(s_xte if j % 2 == 0 else s_xto, 16)

                    xt_load(0)
                    xt_load(1)
                    # coeff plane 0 -> c_all (free once prev add3 done)
                    if it > 0:
                        sp.wait_ge(s_ad, 3 * it)
                    sp.dma_start(out=c_all[:, :], in_=coeffs[0]).then_inc(
                        s_ca, 16
                    )
                    # plane 1 -> stageB (free once prev c3red done)
                    if it > 0:
                        sp.wait_ge(s_c3r, it)
                    sp.dma_start(out=stageB[:, :], in_=coeffs[1]).then_inc(
                        s_cb, 16
                    )
                    # previous iteration's output store: must precede p2/p3
                    # (add3 waits on it) and xt2/xt3 (which wait on pair0,
                    # which is FIFO-after add3 on DVE)
                    if it > 0:
                        sp.wait_ge(s_dr, it)
                        sp.dma_start(
                            out=out_t[:, :], in_=out_all[:, :]
                        ).then_inc(s_out, 16)
                    # planes 2, 3 -> stageB (serialized with the DVE adds;
                    # must precede xt2/xt3: their s_rd gate resolves only
                    # after the adds complete on DVE)
                    sp.wait_ge(s_ad, 3 * it + 1)
                    sp.dma_start(out=stageB[:, :], in_=coeffs[2]).then_inc(
                        s_cb, 16
                    )
                    sp.wait_ge(s_ad, 3 * it + 2)
                    sp.dma_start(out=stageB[:, :], in_=coeffs[3]).then_inc(
                        s_cb, 16
                    )
                    xt_load(2)
                    xt_load(3)
                    xt_load(4)
                    xt_load(5)
                    xt_load(6)
                    xt_load(7)
                    # c3nat -> stageB once add3 consumed the last plane
                    sp.wait_ge(s_ad, 3 * it + 3)
                    sp.dma_start(out=stageB[:, :], in_=c3nat[:, :]).then_inc(
                        s_c3, 16
                    )
                # final output store
                sp.wait_ge(s_dr, NI)
                sp.dma_start(out=out_t[:, :], in_=out_all[:, :]).then_inc(
                    s_out, 16
                )
                sp.wait_ge(s_out, 16 * NI)
                if dummy is not None:
                    sp.dma_start(out=dummy[:, :], in_=bias_sb[:, :]).then_inc(
                        s_fin, 16
                    )
                    sp.wait_ge(s_fin, 113)

            @block.scalar
            def _(act):
                def drains(it):
                    act.wait_ge(s_pe, 8 * (it + 1))
                    act.wait_ge(s_c3r, it + 1)
                    if it > 0:
                        act.wait_ge(s_out, 16 * it)
                    act.activation(
                        out_all[:, :],
                        psum[:, :],
                        ACTF.Identity,
                        bias=bias_sb[:, it % 2 : it % 2 + 1],
                    ).then_inc(s_dr)

                for it in range(NI):
                    for j in range(DC):
                        g = 8 * it + j
                        act.wait_ge(
                            s_xte if j % 2 == 0 else s_xto,
                            16 * (4 * it + j // 2 + 1),
                        )
                        if it == 0:
                            if j in (0, 2, 4):
                                act.wait_ge(s_st0, j // 2 + 1)
                            elif j == 6:
                                act.wait_ge(s_st, 1)
                        elif j == 0:
                            act.wait_ge(s_st, it)
                        if g >= 2 and not (it > 0 and j == 1):
                            act.wait_ge(s_pe, g - 1)  # pow slot recycle
                        sj = s_all[:, j : j + 1]
                        tj = t_all[:, j : j + 1]
                        act.activation(
                            xn_sb[j % 2][:, :], xt_sl[j % 2], ACTF.Relu,
                            bias=tj, scale=sj,
                        ).then_inc(s_xn)
                        act.activation(
                            x2_sb[j % 2][:, :], xt_sl[j % 2], ACTF.Square,
                            bias=tj, scale=sj,
                        ).then_inc(s_xn)
                        if it > 0 and j == 0:
                            drains(it - 1)
                drains(NI - 1)

            @block.vector
            def _(dve):
                if timing_mode:
                    dve.memset(c_all[:, :], 0.3).then_inc(s_fin)
                    dve.wait_ge(s_fin, 1)
                    for t in (xn_sb[0], xn_sb[1], x2_sb[0], x2_sb[1],
                              x3_sb[0], x3_sb[1], out_all):
                        dve.tensor_scalar_mul(t[:, :], c_all[:, :], 1.0)
                    dve.memset(bias_sb[:, :], 0.5)
                    dve.memset(mn_all[:, :], 0.25)
                    dve.memset(mx_all[:, :], 0.75)

                def stats(it, sl, V, fin_sem):
                    # rng=mx-mn; s=1/rng; t=-mn*s  (chained; no prog-order credit)
                    if timing_mode:
                        dve.memset(rng_all[:, sl], 0.25).then_inc(s_dv)
                        dve.wait_ge(s_dv, V + 1)
                        dve.memset(s_all[:, sl], 0.25).then_inc(s_dv)
                        dve.wait_ge(s_dv, V + 2)
                        dve.memset(t0_all[:, sl], 0.25).then_inc(s_dv)
                        dve.wait_ge(s_dv, V + 3)
                        dve.memset(t_all[:, sl], 0.25).then_inc(fin_sem)
                    else:
                        dve.tensor_sub(
                            rng_all[:, sl], mx_all[:, sl], mn_all[:, sl]
                        ).then_inc(s_dv)
                        dve.wait_ge(s_dv, V + 1)
                        dve.reciprocal(s_all[:, sl], rng_all[:, sl]).then_inc(
                            s_dv
                        )
                        dve.wait_ge(s_dv, V + 2)
                        dve.tensor_mul(
                            t0_all[:, sl], mn_all[:, sl], s_all[:, sl]
                        ).then_inc(s_dv)
                        dve.wait_ge(s_dv, V + 3)
                        dve.tensor_scalar_mul(
                            t_all[:, sl], t0_all[:, sl], -1.0
                        ).then_inc(fin_sem)

                def reduce_pair(it, p):
                    c0 = 2 * p
                    dve.tensor_reduce(
                        mn_all[:, c0 : c0 + 2].unsqueeze(2),
                        xt_pair,
                        axis=AX.X,
                        op=ALU.min,
                    ).then_inc(s_rd)
                    dve.tensor_reduce(
                        mx_all[:, c0 : c0 + 2].unsqueeze(2),
                        xt_pair,
                        axis=AX.X,
                        op=ALU.max,
                    ).then_inc(s_rd)

                def x3(it, j):
                    g = 8 * it + j
                    dve.wait_ge(s_xn, 2 * (g + 1))
                    dve.tensor_mul(
                        x3_sb[j % 2][:, :],
                        xn_sb[j % 2][:, :],
                        x2_sb[j % 2][:, :],
                    ).then_inc(s_x3)

                def adds(it):
                    dve.wait_ge(s_ca, 16 * (it + 1))
                    dve.wait_ge(s_cb, 16 * (3 * it + 1))
                    dve.tensor_add(
                        c_all[:, :], c_all[:, :], stageB[:, :]
                    ).then_inc(s_ad)
                    # p2/p3 loads are gated on s_ad, so their arrival (s_cb)
                    # already implies the prior add retired
                    dve.wait_ge(s_cb, 16 * (3 * it + 2))
                    dve.tensor_add(
                        c_all[:, :], c_all[:, :], stageB[:, :]
                    ).then_inc(s_ad)
                    dve.wait_ge(s_cb, 16 * (3 * it + 3))
                    if it > 0:
                        dve.wait_ge(s_out, 16 * it)
                    dve.tensor_add(
                        out_all[:, :], c_all[:, :], stageB[:, :]
                    ).then_inc(s_ad)

                V = 0  # running s_dv count
                for it in range(NI):
                    if it == 0:
                        # per-pair reduces+stats so xn(0) can start early
                        for p in range(4):
                            dve.wait_ge(s_xte, 16 * (p + 1))
                            dve.wait_ge(s_xto, 16 * (p + 1))
                            reduce_pair(0, p)
                            dve.wait_ge(s_rd, 2 * (p + 1))
                            stats(
                                0,
                                slice(2 * p, 2 * p + 2),
                                V,
                                s_st if p == 3 else s_st0,
                            )
                            V += 3
                            if p == 0:
                                adds(0)
                                x3(0, 0)
                                x3(0, 1)
                            else:
                                x3(0, 2 * p)
                                x3(0, 2 * p + 1)
                    else:
                        adds(it)
                        for j in range(DC):
                            x3(it, j)
                            if j % 2 == 1:
                                reduce_pair(it, j // 2)
                        # batched stats for the next iteration (same data)
                        dve.wait_ge(s_rd, 8 * (it + 1))
                        stats(it, slice(0, DC), V, s_st)
                        V += 3
                    # c3 bias reduce (stageB now holds c3nat)
                    dve.wait_ge(s_c3, 16 * (it + 1))
                    if it > 0:
                        dve.wait_ge(s_dr, it)  # bias slot read by it-1 drains
                    dve.tensor_reduce(
                        bias_sb[:, it % 2 : it % 2 + 1],
                        stageB[:, :],
                        axis=AX.X,
                        op=ALU.add,
                    ).then_inc(s_c3r)

            @block.tensor
            def _(pe):
                for it in range(NI):
                    pe.wait_ge(s_ad, 3 * (it + 1))  # coeff tile ready
                    if it > 0:
                        pe.wait_ge(s_dr, it)  # psum drained
                    for j in range(DC):
                        g = 8 * it + j
                        pe.wait_ge(s_x3, g + 1)
                        for k in range(3):  # 0: c0*x3, 1: c1*x2, 2: c2*xn
                            src = [x3_sb, x2_sb, xn_sb][k][j % 2]
                            for q in range(NQ):
                                mm = pe.matmul(
                                    psum[:, q * QW : (q + 1) * QW],
                                    lhsT=c_v[:, k, j, :],
                                    rhs=src[:, q * QW : (q + 1) * QW],
                                    start=(j == 0 and k == 0),
                                    stop=(j == DC - 1 and k == 2),
                                )
                        mm.then_inc(s_pe)

            @block.gpsimd
            def _(pool):
                pass

    return nc


def get_bass(n_iters: int = 1, timing_mode: bool = False) -> bass.Bass:
    key = f"nc{n_iters}_{timing_mode}"
    if key not in _CACHE:
        _CACHE[key] = _build_bass(n_iters, timing_mode)
    return _CACHE[key]


def make_in_maps(x: np.ndarray, spline_coeffs: np.ndarray):
    """Host-side sharding/marshaling only (slicing + transposes, no math)."""
    x = np.ascontiguousarray(np.asarray(x, dtype=np.float32))
    spline_coeffs = np.ascontiguousarray(np.asarray(spline_coeffs, dtype=np.float32))
    xt = np.ascontiguousarray(x.T)  # [D, B]
    in_maps = []
    for r in range(NCORES):
        shard = spline_coeffs[r * OS : (r + 1) * OS]  # [OS, D, S, KC]
        # [s, p, k, j, o] with d = j*128 + p
        a = shard.reshape(OS, DC, P, S, KC).transpose(3, 2, 4, 1, 0)
        in_maps.append(
            {
                "xt": xt,
                "coeffs": np.ascontiguousarray(a).reshape(S, P, KC * DC * OS),
                "c3nat": np.ascontiguousarray(shard[:, :, :, 3]).reshape(
                    OS, D * S
                ),
            }
        )
    return in_maps


def assemble_output(results) -> np.ndarray:
    out = np.concatenate([results[r]["out_t"] for r in range(NCORES)], axis=0)
    return np.ascontiguousarray(out.T)  # [B, O]


def run(x: np.ndarray, spline_coeffs: np.ndarray, trace: bool = False,
        n_iters: int = 1):
    nc = get_bass(n_iters)
    in_maps = make_in_maps(x, spline_coeffs)
    res = run_bass_kernel_spmd(nc, in_maps, list(range(NCORES)), trace=trace)
    return assemble_output(res.results), res


def kernel(x: np.ndarray, spline_coeffs: np.ndarray) -> np.ndarray:
    out, _ = run(x, spline_coeffs, trace=False)
    return out


# revision 3
# speedup vs baseline: 1.2112x; 1.2112x over previous
"""KANLayer kernel v3 for 8 Trainium2 NeuronCores.

Cost model of this environment (measured, see probe*.py):
  * every compute-engine instruction (op OR semaphore wait) costs ~20-50us
    fixed + bytes/(~40-80GB/s); sync-queue instructions ~3us; DMA bandwidth
    ~460GB/s (effectively free at our sizes);
  * >=3 concurrently-active compute engines serialize (par3 probe: PE+DVE+ACT
    independent = sum, not max), so total ~= sum of all engine instruction
    costs; minimizing TOTAL instructions+waits is the whole game;
  * the 192 f32r matmuls (N<=512 ISA cap) are a ~5.2ms floor.

v3 structure (per core, per NEFF iteration):
  * x2 = Square(s*x+t) directly from xt (not from xn) - kills the ACT
    xn->x2 retirement chain;
  * per-feature min/max: [128,2,4096] pair reduces placed AFTER the pair's
    x3 ops, where all their gating is already implied (zero waits);
  * scale/bias stats: ONE batched 4-op chain per iteration, consumed by the
    NEXT iteration (data is identical across NEFF repeats); iteration 0
    computes stats per-pair inline (s_st0);
  * spline coefficient sum: 3 chained DVE adds into the f32r tile the PE
    reads; that tile aliases out_all (dead after last matmul read);
  * c3 bias: c3nat is DMA'd into stageB after the adds consume it; one DVE
    reduce; folded into the single [128,4096] ACT Identity drain;
  * one out-store DMA, delayed one iteration so it never blocks producers.

Sharding: tensor-parallel over O. Core r owns output columns [128r, 128r+128).
"""

import numpy as np

import concourse.bass as bass
import concourse.mybir as mybir
from concourse.bass_utils import run_bass_kernel_spmd

P = 128
B = 4096
D = 1024
O = 1024
S = 4
KC = 4
NCORES = 8
OS = O // NCORES   # 128
DC = D // P        # 8
QW = 512
NQ = B // QW       # 8

F32 = mybir.dt.float32
F32R = mybir.dt.float32r
BF16 = mybir.dt.bfloat16
AX = mybir.AxisListType
ALU = mybir.AluOpType
ACTF = mybir.ActivationFunctionType

_CACHE = {}


def _build_bass(n_iters: int = 1, timing_mode: bool = False) -> bass.Bass:
    nc = bass.Bass(num_devices=NCORES)

    kind = {} if timing_mode else {"kind": "ExternalInput"}
    okind = {} if timing_mode else {"kind": "ExternalOutput"}
    xt = nc.dram_tensor("xt", [D, B], BF16, **kind)
    # [S, P, KC*DC*OS]: plane s is one contiguous [128, 4096] DMA
    coeffs = nc.dram_tensor("coeffs", [S, P, KC * DC * OS], F32, **kind)
    c3nat = nc.dram_tensor("c3nat", [OS, D * S], F32, **kind)
    out_t = nc.dram_tensor("out_t", [OS, B], F32R, **okind)
    dummy = (
        nc.dram_tensor("tout", [P, 2], F32, kind="ExternalOutput")
        if timing_mode
        else None
    )

    from contextlib import ExitStack

    ctx = ExitStack()
    with ctx:
        sem = lambda name: ctx.enter_context(nc.semaphore(name))  # noqa: E731
        s_xte = sem("s_xte")  # +16 per even-chunk xt load (4/iter)
        s_xto = sem("s_xto")  # +16 per odd-chunk xt load (4/iter)
        s_ca = sem("s_ca")    # +16 per c_all plane-0 load
        s_cb = sem("s_cb")    # +16 per stageB plane load (3/iter)
        s_c3 = sem("s_c3")    # +16 per c3nat->stageB load
        s_out = sem("s_out")  # +16 per out store
        s_xn = sem("s_xn")    # +1 per ACT xn AND x2 op (16/iter)
        s_x3 = sem("s_x3")    # +1 per DVE x3 op (8/iter)
        s_rd = sem("s_rd")    # +1 per DVE pair-reduce op (8/iter)
        s_st = sem("s_st")    # +1 per iteration's stats batch
        s_st0 = sem("s_st0")  # iteration-0 per-pair stats groups (3 incs)
        s_ad = sem("s_ad")    # +1 per coeff add (3/iter)
        s_pe = sem("s_pe")    # +1 per finished matmul chunk (8/iter)
        s_dr = sem("s_dr")    # +1 per drain (1/iter)
        s_c3r = sem("s_c3r")  # +1 per c3 bias reduce
        s_dv = sem("s_dv")    # DVE intra-stats retirement chain (3/iter-ish)
        s_fin = sem("s_fin")  # timing-mode init / final store

        sb = lambda name, shape, dtype=F32: ctx.enter_context(  # noqa: E731
            nc.sbuf_tensor(name, shape, dtype)
        )
        xt_sb = sb("xt2", [P, 2 * B], BF16)     # 2 chunk slots, one tile
        xt_sl = [xt_sb[:, i * B : (i + 1) * B] for i in range(2)]
        xt_pair = xt_sb[:, :].rearrange("p (i b) -> p i b", i=2)
        xn_sb = [sb(f"xn{i}", [P, B], F32R) for i in range(2)]
        x2_sb = [sb(f"x2{i}", [P, B], F32R) for i in range(2)]
        x3_sb = [sb(f"x3{i}", [P, B], F32R) for i in range(2)]
        c_all = sb("c_all", [P, KC * DC * OS])  # f32 staging (DVE-only)
        stageB = sb("stageB", [P, KC * DC * OS])  # planes 1-3, then c3nat
        # out_all doubles as the f32r coefficient tile the PE reads (c_v)
        out_all = sb("out_all", [P, B], F32R)
        mn_all = sb("mn_all", [P, DC])
        mx_all = sb("mx_all", [P, DC])
        rng_all = sb("rng_all", [P, DC])
        s_all = sb("s_all", [P, DC])
        t0_all = sb("t0_all", [P, DC])
        t_all = sb("t_all", [P, DC])
        bias_sb = sb("bias_sb", [P, 2])         # ping-pong per iteration

        psum = ctx.enter_context(nc.psum_tensor("ps", [P, B], F32))

        c_v = out_all[:, :].rearrange("p (k j o) -> p k j o", k=KC, j=DC)

        NI = n_iters

        with nc.Block() as block:

            @block.sync
            def _(sp):
                if timing_mode:
                    sp.wait_ge(s_fin, 1)  # c_all memset done
                    zsrc = c_all[:, :]
                    zbf = c_all[:, :].bitcast(BF16)  # finite bf16 bit pattern
                    sp.dma_start(
                        out=xt[:, :].rearrange("(n p) f -> p n f", p=P),
                        in_=bass.AP(
                            tensor=zbf.tensor,
                            offset=zbf.offset,
                            ap=[[zbf.ap[0][0], P], [0, D // P], [1, B]],
                        ),
                    ).then_inc(s_fin, 16)
                    for s in range(S):
                        sp.wait_ge(s_fin, 17 + 16 * s)
                        sp.dma_start(out=coeffs[s], in_=zsrc).then_inc(s_fin, 16)
                    sp.wait_ge(s_fin, 81)
                    sp.dma_start(out=c3nat[:, :], in_=zsrc).then_inc(s_fin, 16)
                    sp.wait_ge(s_fin, 97)
                for it in range(NI):

                    def xt_load(j):
                        g = 8 * it + j
                        if g >= 2:
                            # slot holds chunk g-2: gate on its pair-reduce
                            # AND its xn (x2 reads xt; xn retirement explicit)
                            pr = (
                                8 * it + 2 * ((j - 2) // 2 + 1)
                                if j >= 2
                                else 8 * it
                            )
                            sp.wait_ge(s_rd, pr)
                            sp.wait_ge(s_xn, 2 * (g - 1))
                        sp.dma_start(
                            out=xt_sl[j % 2], in_=xt[j * P : (j + 1) * P, :]
                        ).then_inc(s_xte if j % 2 == 0 else s_xto, 16)

                    xt_load(0)
                    xt_load(1)
                    # coeff plane 0 -> c_all (free once prev add3 done)
                    if it > 0:
                        sp.wait_ge(s_ad, 3 * it)
                    sp.dma_start(out=c_all[:, :], in_=coeffs[0]).then_inc(
                        s_ca, 16
                    )
                    # plane 1 -> stageB (free once prev c3red done)
                    if it > 0:
                        sp.wait_ge(s_c3r, it)
                    sp.dma_start(out=stageB[:, :], in_=coeffs[1]).then_inc(
                        s_cb, 16
                    )
                    # previous iteration's output store: must precede p2/p3
                    # (add3 waits on it) and xt2/xt3 (which wait on pair0,
                    # which is FIFO-after add3 on DVE)
                    if it > 0:
                        sp.wait_ge(s_dr, it)
                        sp.dma_start(
                            out=out_t[:, :], in_=out_all[:, :]
                        ).then_inc(s_out, 16)
                    # planes 2, 3 -> stageB (serialized with the DVE adds;
                    # must precede xt2/xt3: their s_rd gate resolves only
                    # after the adds complete on DVE)
                    sp.wait_ge(s_ad, 3 * it + 1)
                    sp.dma_start(out=stageB[:, :], in_=coeffs[2]).then_inc(
                        s_cb, 16
                    )
                    sp.wait_ge(s_ad, 3 * it + 2)
                    sp.dma_start(out=stageB[:, :], in_=coeffs[3]).then_inc(
                        s_cb, 16
                    )
                    xt_load(2)
                    xt_load(3)
                    xt_load(4)
                    xt_load(5)
                    xt_load(6)
                    xt_load(7)
                    # c3nat -> stageB once add3 consumed the last plane
                    sp.wait_ge(s_ad, 3 * it + 3)
                    sp.dma_start(out=stageB[:, :], in_=c3nat[:, :]).then_inc(
                        s_c3, 16
                    )
                # final output store
                sp.wait_ge(s_dr, NI)
                sp.dma_start(out=out_t[:, :], in_=out_all[:, :]).then_inc(
                    s_out, 16
                )
                sp.wait_ge(s_out, 16 * NI)
                if dummy is not None:
                    sp.dma_start(out=dummy[:, :], in_=bias_sb[:, :]).then_inc(
                        s_fin, 16
                    )
                    sp.wait_ge(s_fin, 113)

            @block.scalar
            def _(act):
                def drains(it):
                    act.wait_ge(s_pe, 8 * (it + 1))
                    act.wait_ge(s_c3r, it + 1)
                    if it > 0:
                        act.wait_ge(s_out, 16 * it)
                    act.activation(
                        out_all[:, :],
                        psum[:, :],
                        ACTF.Identity,
                        bias=bias_sb[:, it % 2 : it % 2 + 1],
                    ).then_inc(s_dr)

                for it in range(NI):
                    for j in range(DC):
                        g = 8 * it + j
                        act.wait_ge(
                            s_xte if j % 2 == 0 else s_xto,
                            16 * (4 * it + j // 2 + 1),
                        )
                        if it == 0:
                            if j in (0, 2, 4):
                                act.wait_ge(s_st0, j // 2 + 1)
                            elif j == 6:
                                act.wait_ge(s_st, 1)
                        elif j == 0:
                            act.wait_ge(s_st, it)
                        if g >= 2 and not (it > 0 and j == 1):
                            act.wait_ge(s_pe, g - 1)  # pow slot recycle
                        sj = s_all[:, j : j + 1]
                        tj = t_all[:, j : j + 1]
                        act.activation(
                            xn_sb[j % 2][:, :], xt_sl[j % 2], ACTF.Relu,
                            bias=tj, scale=sj,
                        ).then_inc(s_xn)
                        act.activation(
                            x2_sb[j % 2][:, :], xt_sl[j % 2], ACTF.Square,
                            bias=tj, scale=sj,
                        ).then_inc(s_xn)
                        if it > 0 and j == 0:
                            drains(it - 1)
                drains(NI - 1)

            @block.vector
            def _(dve):
                if timing_mode:
                    dve.memset(c_all[:, :], 0.3).then_inc(s_fin)
                    dve.wait_ge(s_fin, 1)
                    for t in (xn_sb[0], xn_sb[1], x2_sb[0], x2_sb[1],
                              x3_sb[0], x3_sb[1], out_all):
                        dve.tensor_scalar_mul(t[:, :], c_all[:, :], 1.0)
                    dve.memset(bias_sb[:, :], 0.5)
                    dve.memset(mn_all[:, :], 0.25)
                    dve.memset(mx_all[:, :], 0.75)

                def stats(it, sl, V, fin_sem):
                    # rng=mx-mn; s=1/rng; t=-mn*s  (chained; no prog-order credit)
                    if timing_mode:
                        dve.memset(rng_all[:, sl], 0.25).then_inc(s_dv)
                        dve.wait_ge(s_dv, V + 1)
                        dve.memset(s_all[:, sl], 0.25).then_inc(s_dv)
                        dve.wait_ge(s_dv, V + 2)
                        dve.memset(t0_all[:, sl], 0.25).then_inc(s_dv)
                        dve.wait_ge(s_dv, V + 3)
                        dve.memset(t_all[:, sl], 0.25).then_inc(fin_sem)
                    else:
                        dve.tensor_sub(
                            rng_all[:, sl], mx_all[:, sl], mn_all[:, sl]
                        ).then_inc(s_dv)
                        dve.wait_ge(s_dv, V + 1)
                        dve.reciprocal(s_all[:, sl], rng_all[:, sl]).then_inc(
                            s_dv
                        )
                        dve.wait_ge(s_dv, V + 2)
                        dve.tensor_mul(
                            t0_all[:, sl], mn_all[:, sl], s_all[:, sl]
                        ).then_inc(s_dv)
                        dve.wait_ge(s_dv, V + 3)
                        dve.tensor_scalar_mul(
                            t_all[:, sl], t0_all[:, sl], -1.0
                        ).then_inc(fin_sem)

                def reduce_pair(it, p):
                    c0 = 2 * p
                    dve.tensor_reduce(
                        mn_all[:, c0 : c0 + 2].unsqueeze(2),
                        xt_pair,
                        axis=AX.X,
                        op=ALU.min,
                    ).then_inc(s_rd)
                    dve.tensor_reduce(
                        mx_all[:, c0 : c0 + 2].unsqueeze(2),
                        xt_pair,
                        axis=AX.X,
                        op=ALU.max,
                    ).then_inc(s_rd)

                def x3(it, j):
                    g = 8 * it + j
                    dve.wait_ge(s_xn, 2 * (g + 1))
                    dve.tensor_mul(
                        x3_sb[j % 2][:, :],
                        xn_sb[j % 2][:, :],
                        x2_sb[j % 2][:, :],
                    ).then_inc(s_x3)

                def adds(it):
                    dve.wait_ge(s_ca, 16 * (it + 1))
                    dve.wait_ge(s_cb, 16 * (3 * it + 1))
                    dve.tensor_add(
                        c_all[:, :], c_all[:, :], stageB[:, :]
                    ).then_inc(s_ad)
                    # p2/p3 loads are gated on s_ad, so their arrival (s_cb)
                    # already implies the prior add retired
                    dve.wait_ge(s_cb, 16 * (3 * it + 2))
                    dve.tensor_add(
                        c_all[:, :], c_all[:, :], stageB[:, :]
                    ).then_inc(s_ad)
                    dve.wait_ge(s_cb, 16 * (3 * it + 3))
                    if it > 0:
                        dve.wait_ge(s_out, 16 * it)
                    dve.tensor_add(
                        out_all[:, :], c_all[:, :], stageB[:, :]
                    ).then_inc(s_ad)

                V = 0  # running s_dv count
                for it in range(NI):
                    if it == 0:
                        # per-pair reduces+stats so xn(0) can start early
                        for p in range(4):
                            dve.wait_ge(s_xte, 16 * (p + 1))
                            dve.wait_ge(s_xto, 16 * (p + 1))
                            reduce_pair(0, p)
                            dve.wait_ge(s_rd, 2 * (p + 1))
                            stats(
                                0,
                                slice(2 * p, 2 * p + 2),
                                V,
                                s_st if p == 3 else s_st0,
                            )
                            V += 3
                            if p == 0:
                                adds(0)
                                x3(0, 0)
                                x3(0, 1)
                            else:
                                x3(0, 2 * p)
                                x3(0, 2 * p + 1)
                    else:
                        adds(it)
                        for j in range(DC):
                            x3(it, j)
                            if j % 2 == 1:
                                reduce_pair(it, j // 2)
                        # batched stats for the next iteration (same data)
                        dve.wait_ge(s_rd, 8 * (it + 1))
                        stats(it, slice(0, DC), V, s_st)
                        V += 3
                    # c3 bias reduce (stageB now holds c3nat)
                    dve.wait_ge(s_c3, 16 * (it + 1))
                    if it > 0:
                        dve.wait_ge(s_dr, it)  # bias slot read by it-1 drains
                    dve.tensor_reduce(
                        bias_sb[:, it % 2 : it % 2 + 1],
                        stageB[:, :],
                        axis=AX.X,
                        op=ALU.add,
                    ).then_inc(s_c3r)

            @block.tensor
            def _(pe):
                for it in range(NI):
                    pe.wait_ge(s_ad, 3 * (it + 1))  # coeff tile ready
                    if it > 0:
                        pe.wait_ge(s_dr, it)  # psum drained
                    for j in range(DC):
                        g = 8 * it + j
                        pe.wait_ge(s_x3, g + 1)
                        for k in range(3):  # 0: c0*x3, 1: c1*x2, 2: c2*xn
                            src = [x3_sb, x2_sb, xn_sb][k][j % 2]
                            for q in range(NQ):
                                mm = pe.matmul(
                                    psum[:, q * QW : (q + 1) * QW],
                                    lhsT=c_v[:, k, j, :],
                                    rhs=src[:, q * QW : (q + 1) * QW],
                                    start=(j == 0 and k == 0),
                                    stop=(j == DC - 1 and k == 2),
                                )
                        mm.then_inc(s_pe)

            @block.gpsimd
            def _(pool):
                pass

    return nc


def get_bass(n_iters: int = 1, timing_mode: bool = False) -> bass.Bass:
    key = f"nc{n_iters}_{timing_mode}"
    if key not in _CACHE:
        _CACHE[key] = _build_bass(n_iters, timing_mode)
    return _CACHE[key]


def make_in_maps(x: np.ndarray, spline_coeffs: np.ndarray):
    """Host-side sharding/marshaling only (slicing + transposes, no math)."""
    import ml_dtypes

    x = np.ascontiguousarray(np.asarray(x, dtype=np.float32))
    spline_coeffs = np.ascontiguousarray(np.asarray(spline_coeffs, dtype=np.float32))
    xt = np.ascontiguousarray(x.T.astype(ml_dtypes.bfloat16))  # [D, B] bf16
    in_maps = []
    for r in range(NCORES):
        shard = spline_coeffs[r * OS : (r + 1) * OS]  # [OS, D, S, KC]
        # [s, p, k, j, o] with d = j*128 + p
        a = shard.reshape(OS, DC, P, S, KC).transpose(3, 2, 4, 1, 0)
        in_maps.append(
            {
                "xt": xt,
                "coeffs": np.ascontiguousarray(a).reshape(S, P, KC * DC * OS),
                "c3nat": np.ascontiguousarray(shard[:, :, :, 3]).reshape(
                    OS, D * S
                ),
            }
        )
    return in_maps


def assemble_output(results) -> np.ndarray:
    out = np.concatenate([results[r]["out_t"] for r in range(NCORES)], axis=0)
    return np.ascontiguousarray(out.T)  # [B, O]


def run(x: np.ndarray, spline_coeffs: np.ndarray, trace: bool = False,
        n_iters: int = 1):
    nc = get_bass(n_iters)
    in_maps = make_in_maps(x, spline_coeffs)
    res = run_bass_kernel_spmd(nc, in_maps, list(range(NCORES)), trace=trace)
    return assemble_output(res.results), res


def kernel(x: np.ndarray, spline_coeffs: np.ndarray) -> np.ndarray:
    out, _ = run(x, spline_coeffs, trace=False)
    return out
